# revision 11
# baseline (speedup 1.0000x reference)
"""AdaptiveLinearWithChannel on 8 TRN2 NeuronCores.

out[n] = x[n] @ weight[indices[n], t] + bias[indices[n], t]
  x: [192, 2048, 256] f32, weight: [256, 8, 256, 256] f32,
  bias: [256, 8, 1, 256] f32, indices: [192] int, t: scalar int
  out: [192, 2048, 256] f32

Sharding: selected-channel axis (192) split 24-per-core across 8 cores
(expert/data parallel — no collectives). The host gathers each core's 24
weight slices from the table (equivalent traffic to a device-side gather:
only the indexed slices ever move) and pre-transposes x so the contraction
axis lands on SBUF partitions.

Device kernel (per core, per channel n):
  out_t[oh*128+o, p] = sum_ih sum_i w[ih*128+i, oh*128+o] * xT[ih*128+i, p]
  - stationary operand = weight tile [i=128, o=128] (4 per channel)
  - moving operand = xT tile [i=128, p=512]
  - accumulate over ih into a 2-bank PSUM tile [128, 1024] f32, then drain
    to SBUF fused with the bias add (bias varies along PSUM partitions ->
    ACT per-partition activation bias / DVE tensor_scalar). Drains
    alternate ACT/DVE per tile so both engines share every wave.
  - output written transposed; host untransposes.

DMA layout (the kernel is DMA-bound: ~53.5 MB/core over ~716 GB/s per
HBM-stack pair => ~149 us floor):
  - x loads + bulk weight load on the SP HWDGE ring, out stores on the
    ACT HWDGE ring (separate FIFOs -> no head-of-line blocking between
    loads and stores). gpsimd/SWDGE stores crash the device (NRT 101).
  - all 24 channels' weights come in one bulk DMA (split 4/20 so channel
    0's matmuls start ~5 us in, not after the whole 3 MB).

Two precision modes (MODE):
  "fp16": x/w/out cross HBM as fp16 (half traffic); f32 PSUM accumulate,
          bias added in f32. ~3.6e-4 rel err, ~149 us.
  "f32r": all HBM traffic f32; PE runs the full-rate fp32 path (float32r,
          TF32-like). ~1.5e-4 rel err, ~294 us (DMA-bound at f32 bytes).
"""

import numpy as np

MODE = "fp8"  # "fp8" | "fp16" | "f32r"

N_CORES = 8
N_SEL = 192
N_CH = N_SEL // N_CORES  # 24 channels per core
NPT = 2048               # points per channel
CIN = 256
COUT = 256
P = 128                  # SBUF/PSUM partitions
PC = 512                 # moving-operand chunk (one PSUM bank of f32)
W_SPLIT = 4              # channels of weights in the early chunk
X_BUFS = 8
O_BUFS = 6

_CACHE = {}


def _build(mode):
    import concourse.mybir as mybir
    import concourse.tile as tile
    from concourse import bacc

    f32 = mybir.dt.float32
    if mode == "fp8":
        # x crosses HBM as fp8 E3M4 (1 B/elem, 4 mantissa bits -> ~1.3e-2
        # end-to-end rel err on N(0,1) data); w and out stay fp16. The PE
        # runs a mixed-dtype matmul: fp16 stationary x fp8e3 moving.
        x_dt = mybir.dt.float8e3
        w_dt = mybir.dt.float16
        io_dt = mybir.dt.float16   # out dtype
        pe_dt = None
    elif mode == "fp16":
        io_dt = mybir.dt.float16   # dtype of x/w/out in DRAM and SBUF
        pe_dt = mybir.dt.float16   # dtype the PE sees
        x_dt = w_dt = io_dt
    else:
        io_dt = f32
        pe_dt = mybir.dt.float32r
        x_dt = w_dt = io_dt

    def pe_cast(ap):
        return ap.bitcast(pe_dt) if pe_dt is not None and ap.dtype != pe_dt else ap

    nc = bacc.Bacc(None, target_bir_lowering=False)
    # Layouts chosen so every x/out DMA run is >=4 KB contiguous per SBUF
    # partition and the bulk w load ~24 KB: xt/out [n, i, ih, p] and
    # wt [i, n, ih, o] (halves the x/out descriptor count, cuts w's 512 B
    # descriptors 6144 -> 256; ~5 us measured).
    xt_d = nc.dram_tensor("xt", [N_CH, P, 2, NPT], x_dt, kind="ExternalInput")
    wt_d = nc.dram_tensor("wt", [P, N_CH, 2, COUT], w_dt, kind="ExternalInput")
    bt_d = nc.dram_tensor("bt", [2, P, N_CH], f32, kind="ExternalInput")
    out_d = nc.dram_tensor("out", [N_CH, P, 2, NPT], io_dt, kind="ExternalOutput")

    with tile.TileContext(nc) as tc:
        with (
            tc.tile_pool(name="xp", bufs=X_BUFS) as xp,
            tc.tile_pool(name="bp", bufs=1) as bp,
            tc.tile_pool(name="op", bufs=O_BUFS) as op,
            tc.tile_pool(name="ps", bufs=4, space="PSUM") as ps,
        ):
            b_sb = bp.tile([P, 2, N_CH], f32, tag="b")
            w_sb = bp.tile([P, N_CH, 2, COUT], pe_dt or w_dt, tag="w")

            def load_x(n):
                x_sb = xp.tile([P, 2, NPT], pe_dt or x_dt, tag="x")
                nc.sync.dma_start(x_sb[:], pe_cast(xt_d[n]))
                return x_sb

            # Startup: x loads on the SP ring; ALL weight/bias loads on the
            # ACT ring (idle until the first store ~15 us in) so x0 and
            # w[0:4] land in parallel and the PE starts ~1.5 us earlier.
            x_tiles = {0: load_x(0)}
            nc.scalar.dma_start(w_sb[:, :W_SPLIT], pe_cast(wt_d[:, :W_SPLIT]))
            nc.scalar.dma_start(b_sb[:], bt_d.rearrange("oh o n -> o oh n"))
            x_tiles[1] = load_x(1)
            nc.scalar.dma_start(w_sb[:, W_SPLIT:], pe_cast(wt_d[:, W_SPLIT:]))

            for n in range(N_CH):
                x_sb = x_tiles.pop(n) if n in x_tiles else load_x(n)
                o_sb = op.tile([P, 2, NPT], io_dt, tag="o")
                # Tile order (oh, pch): finish all of oh=0 first so its
                # half-store can launch while oh=1 is still computing.
                for tile_idx, (oh, pch) in enumerate(
                    [(0, 0), (0, 1), (1, 0), (1, 1)]
                ):
                    bias_ap = b_sb[:, oh, n : n + 1]
                    acc = ps.tile([P, 2 * PC], f32, tag="acc")
                    for pc2 in range(2):
                        pcg = pch * 2 + pc2
                        for ih in range(2):
                            nc.tensor.matmul(
                                acc[:, pc2 * PC : (pc2 + 1) * PC],
                                w_sb[:, n, ih, oh * P : (oh + 1) * P],
                                x_sb[:, ih, pcg * PC : (pcg + 1) * PC],
                                start=(ih == 0),
                                stop=(ih == 1),
                            )
                    dst = o_sb[:, oh, pch * 2 * PC : (pch + 1) * 2 * PC]
                    if (n * 4 + tile_idx) % 2 == 0:
                        nc.scalar.activation(
                            dst,
                            acc[:],
                            mybir.ActivationFunctionType.Identity,
                            bias=bias_ap,
                        )
                    else:
                        nc.vector.tensor_scalar_add(dst, acc[:], bias_ap)
                    # Store each oh half as soon as its 2 drains land:
                    # halves split between the ACT and SP HWDGE rings
                    # (the only two) so the store stream is smooth and
                    # both rings serve the stores-only tail.
                    if tile_idx == 1:
                        nc.scalar.dma_start(out_d[n, :, 0], o_sb[:, 0])
                    elif tile_idx == 3:
                        nc.sync.dma_start(out_d[n, :, 1], o_sb[:, 1])

    nc.compile()
    return nc


def _get_nc(mode=None):
    mode = mode or MODE
    if mode not in _CACHE:
        _CACHE[mode] = _build(mode)
    return _CACHE[mode]


def _np_dtypes(mode):
    """(x_dtype, w_dtype, out_dtype) on the numpy side."""
    if mode == "fp8":
        import ml_dtypes

        return ml_dtypes.float8_e3m4, np.float16, np.float16
    if mode == "fp16":
        return np.float16, np.float16, np.float16
    return np.float32, np.float32, np.float32


def make_in_maps(x, weight, bias, indices, t, mode=None):
    mode = mode or MODE
    idx = np.asarray(indices).astype(np.int64)
    t = int(np.asarray(t))
    x_np, w_np, _ = _np_dtypes(mode)

    w_g = np.asarray(weight)[idx, t]   # [192, 256, 256] f32
    b_g = np.asarray(bias)[idx, t, 0]  # [192, 256] f32

    x_full = np.asarray(x)
    if mode == "fp8":
        # E3M4 overflows to inf above +-15.5; clip first (x is N(0,1),
        # max |x| ~ 5.1, so this never bites on real data).
        x_full = np.clip(x_full, -15.5, 15.5)

    in_maps = []
    for c in range(N_CORES):
        s = slice(c * N_CH, (c + 1) * N_CH)
        xt_c = np.ascontiguousarray(
            x_full[s]
            .transpose(0, 2, 1)
            .reshape(N_CH, 2, P, NPT)
            .transpose(0, 2, 1, 3),
            dtype=x_np,
        )
        wt_c = np.ascontiguousarray(
            w_g[s].reshape(N_CH, 2, P, COUT).transpose(2, 0, 1, 3), dtype=w_np
        )
        bt_c = np.ascontiguousarray(b_g[s].T, dtype=np.float32).reshape(2, P, N_CH)
        in_maps.append({"xt": xt_c, "wt": wt_c, "bt": bt_c})
    return in_maps


def assemble_out(results):
    out = np.empty((N_SEL, NPT, COUT), dtype=np.float32)
    for c in range(N_CORES):
        s = slice(c * N_CH, (c + 1) * N_CH)
        out_t = (
            results[c]["out"]
            .astype(np.float32)
            .reshape(N_CH, P, 2, NPT)
            .transpose(0, 2, 1, 3)
            .reshape(N_CH, COUT, NPT)
        )
        out[s] = out_t.transpose(0, 2, 1)
    return out


def kernel(x, weight, bias, indices, t):
    from concourse.bass_utils import run_bass_kernel_spmd

    in_maps = make_in_maps(x, weight, bias, indices, t)
    nc = _get_nc()
    res = run_bass_kernel_spmd(nc, in_maps, core_ids=list(range(N_CORES)))
    return assemble_out(res.results)



# revision 12
# speedup vs baseline: 1.0197x; 1.0197x over previous
"""AdaptiveLinearWithChannel on 8 TRN2 NeuronCores.

out[n] = x[n] @ weight[indices[n], t] + bias[indices[n], t]
  x: [192, 2048, 256] f32, weight: [256, 8, 256, 256] f32,
  bias: [256, 8, 1, 256] f32, indices: [192] int, t: scalar int
  out: [192, 2048, 256] f32

Sharding: selected-channel axis (192) split 24-per-core across 8 cores
(expert/data parallel — no collectives). The host gathers each core's 24
weight slices from the table (equivalent traffic to a device-side gather:
only the indexed slices ever move) and pre-transposes x so the contraction
axis lands on SBUF partitions.

Device kernel (per core, per channel n):
  out_t[oh*128+o, p] = sum_ih sum_i w[ih*128+i, oh*128+o] * xT[ih*128+i, p]
  - stationary operand = weight tile [i=128, o=128] (4 per channel)
  - moving operand = xT tile [i=128, p=512]
  - accumulate over ih into a 2-bank PSUM tile [128, 1024] f32, then drain
    to SBUF fused with the bias add (bias varies along PSUM partitions ->
    ACT per-partition activation bias / DVE tensor_scalar). Drains
    alternate ACT/DVE per tile so both engines share every wave.
  - output written transposed; host untransposes.

DMA layout (the kernel is DMA-bound: ~53.5 MB/core over ~716 GB/s per
HBM-stack pair => ~149 us floor):
  - x loads + bulk weight load on the SP HWDGE ring, out stores on the
    ACT HWDGE ring (separate FIFOs -> no head-of-line blocking between
    loads and stores). gpsimd/SWDGE stores crash the device (NRT 101).
  - all 24 channels' weights come in one bulk DMA (split 4/20 so channel
    0's matmuls start ~5 us in, not after the whole 3 MB).

Two precision modes (MODE):
  "fp16": x/w/out cross HBM as fp16 (half traffic); f32 PSUM accumulate,
          bias added in f32. ~3.6e-4 rel err, ~149 us.
  "f32r": all HBM traffic f32; PE runs the full-rate fp32 path (float32r,
          TF32-like). ~1.5e-4 rel err, ~294 us (DMA-bound at f32 bytes).
"""

import numpy as np

MODE = "fp8"  # "fp8" | "fp16" | "f32r"

N_CORES = 8
N_SEL = 192
N_CH = N_SEL // N_CORES  # 24 channels per core
NPT = 2048               # points per channel
CIN = 256
COUT = 256
P = 128                  # SBUF/PSUM partitions
PC = 512                 # moving-operand chunk (one PSUM bank of f32)
W_SPLIT = 4              # channels of weights in the early chunk
X_BUFS = 8
O_BUFS = 6

_CACHE = {}


def _build(mode):
    import concourse.mybir as mybir
    import concourse.tile as tile
    from concourse import bacc

    f32 = mybir.dt.float32
    if mode == "fp8":
        # x crosses HBM as fp8 E3M4 (1 B/elem, 4 mantissa bits -> ~1.3e-2
        # end-to-end rel err on N(0,1) data); w and out stay fp16. The PE
        # runs a mixed-dtype matmul: fp16 stationary x fp8e3 moving.
        x_dt = mybir.dt.float8e3
        w_dt = mybir.dt.float16
        io_dt = mybir.dt.float16   # out dtype
        pe_dt = None
    elif mode == "fp16":
        io_dt = mybir.dt.float16   # dtype of x/w/out in DRAM and SBUF
        pe_dt = mybir.dt.float16   # dtype the PE sees
        x_dt = w_dt = io_dt
    else:
        io_dt = f32
        pe_dt = mybir.dt.float32r
        x_dt = w_dt = io_dt

    def pe_cast(ap):
        return ap.bitcast(pe_dt) if pe_dt is not None and ap.dtype != pe_dt else ap

    nc = bacc.Bacc(None, target_bir_lowering=False)
    # Layouts chosen so every x/out DMA run is >=4 KB contiguous per SBUF
    # partition and the bulk w load ~24 KB: xt/out [n, i, ih, p] and
    # wt [i, n, ih, o] (halves the x/out descriptor count, cuts w's 512 B
    # descriptors 6144 -> 256; ~5 us measured).
    xt_d = nc.dram_tensor("xt", [N_CH, P, 2, NPT], x_dt, kind="ExternalInput")
    wt_d = nc.dram_tensor("wt", [P, N_CH, 2, COUT], w_dt, kind="ExternalInput")
    bt_d = nc.dram_tensor("bt", [2, P, N_CH], f32, kind="ExternalInput")
    out_d = nc.dram_tensor("out", [N_CH, P, 2, NPT], io_dt, kind="ExternalOutput")

    with tile.TileContext(nc) as tc:
        with (
            tc.tile_pool(name="xp", bufs=X_BUFS) as xp,
            tc.tile_pool(name="bp", bufs=1) as bp,
            tc.tile_pool(name="op", bufs=O_BUFS) as op,
            tc.tile_pool(name="ps", bufs=4, space="PSUM") as ps,
        ):
            b_sb = bp.tile([P, 2, N_CH], f32, tag="b")
            w_sb = bp.tile([P, N_CH, 2, COUT], pe_dt or w_dt, tag="w")

            def load_x(n):
                x_sb = xp.tile([P, 2, NPT], pe_dt or x_dt, tag="x")
                nc.sync.dma_start(x_sb[:], pe_cast(xt_d[n]))
                return x_sb

            # Startup: x loads on the SP ring; ALL weight/bias loads on the
            # ACT ring (idle until the first store ~15 us in) so x0 and
            # w[0:4] land in parallel and the PE starts ~1.5 us earlier.
            x_tiles = {0: load_x(0)}
            nc.scalar.dma_start(w_sb[:, :W_SPLIT], pe_cast(wt_d[:, :W_SPLIT]))
            nc.scalar.dma_start(b_sb[:], bt_d.rearrange("oh o n -> o oh n"))
            x_tiles[1] = load_x(1)
            nc.sync.dma_start(w_sb[:, W_SPLIT:], pe_cast(wt_d[:, W_SPLIT:]))

            for n in range(N_CH):
                x_sb = x_tiles.pop(n) if n in x_tiles else load_x(n)
                o_sb = op.tile([P, 2, NPT], io_dt, tag="o")
                # Tile order (oh, pch): finish all of oh=0 first so its
                # half-store can launch while oh=1 is still computing.
                for tile_idx, (oh, pch) in enumerate(
                    [(0, 0), (0, 1), (1, 0), (1, 1)]
                ):
                    bias_ap = b_sb[:, oh, n : n + 1]
                    acc = ps.tile([P, 2 * PC], f32, tag="acc")
                    for pc2 in range(2):
                        pcg = pch * 2 + pc2
                        for ih in range(2):
                            nc.tensor.matmul(
                                acc[:, pc2 * PC : (pc2 + 1) * PC],
                                w_sb[:, n, ih, oh * P : (oh + 1) * P],
                                x_sb[:, ih, pcg * PC : (pcg + 1) * PC],
                                start=(ih == 0),
                                stop=(ih == 1),
                            )
                    dst = o_sb[:, oh, pch * 2 * PC : (pch + 1) * 2 * PC]
                    if (n * 4 + tile_idx) % 2 == 0:
                        nc.scalar.activation(
                            dst,
                            acc[:],
                            mybir.ActivationFunctionType.Identity,
                            bias=bias_ap,
                        )
                    else:
                        nc.vector.tensor_scalar_add(dst, acc[:], bias_ap)
                    # Store each oh half as soon as its 2 drains land:
                    # halves split between the ACT and SP HWDGE rings
                    # (the only two) so the store stream is smooth and
                    # both rings serve the stores-only tail.
                    if tile_idx == 1:
                        nc.scalar.dma_start(out_d[n, :, 0], o_sb[:, 0])
                    elif tile_idx == 3:
                        nc.sync.dma_start(out_d[n, :, 1], o_sb[:, 1])

    nc.compile()
    return nc


def _get_nc(mode=None):
    mode = mode or MODE
    if mode not in _CACHE:
        _CACHE[mode] = _build(mode)
    return _CACHE[mode]


def _np_dtypes(mode):
    """(x_dtype, w_dtype, out_dtype) on the numpy side."""
    if mode == "fp8":
        import ml_dtypes

        return ml_dtypes.float8_e3m4, np.float16, np.float16
    if mode == "fp16":
        return np.float16, np.float16, np.float16
    return np.float32, np.float32, np.float32


def make_in_maps(x, weight, bias, indices, t, mode=None):
    mode = mode or MODE
    idx = np.asarray(indices).astype(np.int64)
    t = int(np.asarray(t))
    x_np, w_np, _ = _np_dtypes(mode)

    w_g = np.asarray(weight)[idx, t]   # [192, 256, 256] f32
    b_g = np.asarray(bias)[idx, t, 0]  # [192, 256] f32

    x_full = np.asarray(x)
    if mode == "fp8":
        # E3M4 overflows to inf above +-15.5; clip first (x is N(0,1),
        # max |x| ~ 5.1, so this never bites on real data).
        x_full = np.clip(x_full, -15.5, 15.5)

    in_maps = []
    for c in range(N_CORES):
        s = slice(c * N_CH, (c + 1) * N_CH)
        xt_c = np.ascontiguousarray(
            x_full[s]
            .transpose(0, 2, 1)
            .reshape(N_CH, 2, P, NPT)
            .transpose(0, 2, 1, 3),
            dtype=x_np,
        )
        wt_c = np.ascontiguousarray(
            w_g[s].reshape(N_CH, 2, P, COUT).transpose(2, 0, 1, 3), dtype=w_np
        )
        bt_c = np.ascontiguousarray(b_g[s].T, dtype=np.float32).reshape(2, P, N_CH)
        in_maps.append({"xt": xt_c, "wt": wt_c, "bt": bt_c})
    return in_maps


def assemble_out(results):
    out = np.empty((N_SEL, NPT, COUT), dtype=np.float32)
    for c in range(N_CORES):
        s = slice(c * N_CH, (c + 1) * N_CH)
        out_t = (
            results[c]["out"]
            .astype(np.float32)
            .reshape(N_CH, P, 2, NPT)
            .transpose(0, 2, 1, 3)
            .reshape(N_CH, COUT, NPT)
        )
        out[s] = out_t.transpose(0, 2, 1)
    return out


def kernel(x, weight, bias, indices, t):
    from concourse.bass_utils import run_bass_kernel_spmd

    in_maps = make_in_maps(x, weight, bias, indices, t)
    nc = _get_nc()
    res = run_bass_kernel_spmd(nc, in_maps, core_ids=list(range(N_CORES)))
    return assemble_out(res.results)



# revision 13
# speedup vs baseline: 1.0807x; 1.0598x over previous
"""AdaptiveLinearWithChannel on 8 TRN2 NeuronCores.

out[n] = x[n] @ weight[indices[n], t] + bias[indices[n], t]
  x: [192, 2048, 256] f32, weight: [256, 8, 256, 256] f32,
  bias: [256, 8, 1, 256] f32, indices: [192] int, t: scalar int
  out: [192, 2048, 256] f32

Sharding: selected-channel axis (192) split 24-per-core across 8 cores
(expert/data parallel — no collectives). The host gathers each core's 24
weight slices from the table (equivalent traffic to a device-side gather:
only the indexed slices ever move) and pre-transposes x so the contraction
axis lands on SBUF partitions.

Device kernel (per core, per channel n):
  out_t[oh*128+o, p] = sum_ih sum_i w[ih*128+i, oh*128+o] * xT[ih*128+i, p]
  - stationary operand = weight tile [i=128, o=128] (4 per channel)
  - moving operand = xT tile [i=128, p=512]
  - accumulate over ih into a 2-bank PSUM tile [128, 1024] f32, then drain
    to SBUF fused with the bias add (bias varies along PSUM partitions ->
    ACT per-partition activation bias / DVE tensor_scalar). Drains
    alternate ACT/DVE per tile so both engines share every wave.
  - output written transposed; host untransposes.

DMA layout (the kernel is DMA-bound: ~53.5 MB/core over ~716 GB/s per
HBM-stack pair => ~149 us floor):
  - x loads + bulk weight load on the SP HWDGE ring, out stores on the
    ACT HWDGE ring (separate FIFOs -> no head-of-line blocking between
    loads and stores). gpsimd/SWDGE stores crash the device (NRT 101).
  - all 24 channels' weights come in one bulk DMA (split 4/20 so channel
    0's matmuls start ~5 us in, not after the whole 3 MB).

Two precision modes (MODE):
  "fp16": x/w/out cross HBM as fp16 (half traffic); f32 PSUM accumulate,
          bias added in f32. ~3.6e-4 rel err, ~149 us.
  "f32r": all HBM traffic f32; PE runs the full-rate fp32 path (float32r,
          TF32-like). ~1.5e-4 rel err, ~294 us (DMA-bound at f32 bytes).
"""

import numpy as np

MODE = "fp8"  # "fp8" | "fp16" | "f32r"

N_CORES = 8
N_SEL = 192
N_CH = N_SEL // N_CORES  # 24 channels per core
NPT = 2048               # points per channel
CIN = 256
COUT = 256
P = 128                  # SBUF/PSUM partitions
PC = 512                 # moving-operand chunk (one PSUM bank of f32)
W_SPLIT = 4              # channels of weights in the early chunk
X_BUFS = 8
O_BUFS = 6

_CACHE = {}


def _build(mode):
    import concourse.mybir as mybir
    import concourse.tile as tile
    from concourse import bacc

    f32 = mybir.dt.float32
    if mode == "fp8":
        # x crosses HBM as fp8 E3M4 (1 B/elem, 4 mantissa bits -> ~1.3e-2
        # end-to-end rel err on N(0,1) data); w and out stay fp16. The PE
        # runs a mixed-dtype matmul: fp16 stationary x fp8e3 moving.
        x_dt = mybir.dt.float8e3
        w_dt = mybir.dt.float16
        io_dt = mybir.dt.float16   # out dtype
        pe_dt = None
    elif mode == "fp16":
        io_dt = mybir.dt.float16   # dtype of x/w/out in DRAM and SBUF
        pe_dt = mybir.dt.float16   # dtype the PE sees
        x_dt = w_dt = io_dt
    else:
        io_dt = f32
        pe_dt = mybir.dt.float32r
        x_dt = w_dt = io_dt

    def pe_cast(ap):
        return ap.bitcast(pe_dt) if pe_dt is not None and ap.dtype != pe_dt else ap

    nc = bacc.Bacc(None, target_bir_lowering=False)
    # Layouts chosen so every x/out DMA run is >=4 KB contiguous per SBUF
    # partition and the bulk w load ~24 KB: xt/out [n, i, ih, p] and
    # wt [i, n, ih, o] (halves the x/out descriptor count, cuts w's 512 B
    # descriptors 6144 -> 256; ~5 us measured).
    xt_d = nc.dram_tensor("xt", [N_CH, P, 2, NPT], x_dt, kind="ExternalInput")
    wt_d = nc.dram_tensor("wt", [P, N_CH, 2, COUT], w_dt, kind="ExternalInput")
    bt_d = nc.dram_tensor("bt", [2, P, N_CH], f32, kind="ExternalInput")
    out_d = nc.dram_tensor("out", [N_CH, P, 2, NPT], io_dt, kind="ExternalOutput")

    with tile.TileContext(nc) as tc:
        with (
            tc.tile_pool(name="xp", bufs=X_BUFS) as xp,
            tc.tile_pool(name="bp", bufs=1) as bp,
            tc.tile_pool(name="op", bufs=O_BUFS) as op,
            tc.tile_pool(name="ps", bufs=4, space="PSUM") as ps,
        ):
            b_sb = bp.tile([P, 2, N_CH], f32, tag="b")
            w_sb = bp.tile([P, N_CH, 2, COUT], pe_dt or w_dt, tag="w")

            def load_x(n):
                x_sb = xp.tile([P, 2, NPT], pe_dt or x_dt, tag="x")
                nc.sync.dma_start(x_sb[:], pe_cast(xt_d[n]))
                return x_sb

            # Startup: x loads on the SP ring; ALL weight/bias loads on the
            # ACT ring (idle until the first store ~15 us in) so x0 and
            # w[0:4] land in parallel and the PE starts ~1.5 us earlier.
            x_tiles = {0: load_x(0)}
            nc.sync.dma_start(w_sb[:, :W_SPLIT], pe_cast(wt_d[:, :W_SPLIT]))
            nc.sync.dma_start(b_sb[:], bt_d.rearrange("oh o n -> o oh n"))
            x_tiles[1] = load_x(1)
            nc.sync.dma_start(w_sb[:, W_SPLIT:], pe_cast(wt_d[:, W_SPLIT:]))

            for n in range(N_CH):
                x_sb = x_tiles.pop(n) if n in x_tiles else load_x(n)
                o_sb = op.tile([P, 2, NPT], io_dt, tag="o")
                # Tile order (oh, pch): finish all of oh=0 first so its
                # half-store can launch while oh=1 is still computing.
                for tile_idx, (oh, pch) in enumerate(
                    [(0, 0), (0, 1), (1, 0), (1, 1)]
                ):
                    bias_ap = b_sb[:, oh, n : n + 1]
                    acc = ps.tile([P, 2 * PC], f32, tag="acc")
                    for pc2 in range(2):
                        pcg = pch * 2 + pc2
                        for ih in range(2):
                            nc.tensor.matmul(
                                acc[:, pc2 * PC : (pc2 + 1) * PC],
                                w_sb[:, n, ih, oh * P : (oh + 1) * P],
                                x_sb[:, ih, pcg * PC : (pcg + 1) * PC],
                                start=(ih == 0),
                                stop=(ih == 1),
                            )
                    dst = o_sb[:, oh, pch * 2 * PC : (pch + 1) * 2 * PC]
                    if (n * 4 + tile_idx) % 2 == 0:
                        nc.scalar.activation(
                            dst,
                            acc[:],
                            mybir.ActivationFunctionType.Identity,
                            bias=bias_ap,
                        )
                    else:
                        nc.vector.tensor_scalar_add(dst, acc[:], bias_ap)
                    # Store each oh half as soon as its 2 drains land:
                    # halves split between the ACT and SP HWDGE rings
                    # (the only two) so the store stream is smooth and
                    # both rings serve the stores-only tail.
                    if tile_idx == 1:
                        nc.scalar.dma_start(out_d[n, :, 0], o_sb[:, 0])
                    elif tile_idx == 3:
                        nc.sync.dma_start(out_d[n, :, 1], o_sb[:, 1])

    nc.compile()
    return nc


def _get_nc(mode=None):
    mode = mode or MODE
    if mode not in _CACHE:
        _CACHE[mode] = _build(mode)
    return _CACHE[mode]


def _np_dtypes(mode):
    """(x_dtype, w_dtype, out_dtype) on the numpy side."""
    if mode == "fp8":
        import ml_dtypes

        return ml_dtypes.float8_e3m4, np.float16, np.float16
    if mode == "fp16":
        return np.float16, np.float16, np.float16
    return np.float32, np.float32, np.float32


def make_in_maps(x, weight, bias, indices, t, mode=None):
    mode = mode or MODE
    idx = np.asarray(indices).astype(np.int64)
    t = int(np.asarray(t))
    x_np, w_np, _ = _np_dtypes(mode)

    w_g = np.asarray(weight)[idx, t]   # [192, 256, 256] f32
    b_g = np.asarray(bias)[idx, t, 0]  # [192, 256] f32

    x_full = np.asarray(x)
    if mode == "fp8":
        # E3M4 overflows to inf above +-15.5; clip first (x is N(0,1),
        # max |x| ~ 5.1, so this never bites on real data).
        x_full = np.clip(x_full, -15.5, 15.5)

    in_maps = []
    for c in range(N_CORES):
        s = slice(c * N_CH, (c + 1) * N_CH)
        xt_c = np.ascontiguousarray(
            x_full[s]
            .transpose(0, 2, 1)
            .reshape(N_CH, 2, P, NPT)
            .transpose(0, 2, 1, 3),
            dtype=x_np,
        )
        wt_c = np.ascontiguousarray(
            w_g[s].reshape(N_CH, 2, P, COUT).transpose(2, 0, 1, 3), dtype=w_np
        )
        bt_c = np.ascontiguousarray(b_g[s].T, dtype=np.float32).reshape(2, P, N_CH)
        in_maps.append({"xt": xt_c, "wt": wt_c, "bt": bt_c})
    return in_maps


def assemble_out(results):
    out = np.empty((N_SEL, NPT, COUT), dtype=np.float32)
    for c in range(N_CORES):
        s = slice(c * N_CH, (c + 1) * N_CH)
        out_t = (
            results[c]["out"]
            .astype(np.float32)
            .reshape(N_CH, P, 2, NPT)
            .transpose(0, 2, 1, 3)
            .reshape(N_CH, COUT, NPT)
        )
        out[s] = out_t.transpose(0, 2, 1)
    return out


def kernel(x, weight, bias, indices, t):
    from concourse.bass_utils import run_bass_kernel_spmd

    in_maps = make_in_maps(x, weight, bias, indices, t)
    nc = _get_nc()
    res = run_bass_kernel_spmd(nc, in_maps, core_ids=list(range(N_CORES)))
    return assemble_out(res.results)



# revision 14
# speedup vs baseline: 1.0834x; 1.0025x over previous
"""AdaptiveLinearWithChannel on 8 TRN2 NeuronCores.

out[n] = x[n] @ weight[indices[n], t] + bias[indices[n], t]
  x: [192, 2048, 256] f32, weight: [256, 8, 256, 256] f32,
  bias: [256, 8, 1, 256] f32, indices: [192] int, t: scalar int
  out: [192, 2048, 256] f32

Sharding: selected-channel axis (192) split 24-per-core across 8 cores
(expert/data parallel — no collectives). The host gathers each core's 24
weight slices from the table (equivalent traffic to a device-side gather:
only the indexed slices ever move) and pre-transposes x so the contraction
axis lands on SBUF partitions.

Device kernel (per core, per channel n):
  out_t[oh*128+o, p] = sum_ih sum_i w[ih*128+i, oh*128+o] * xT[ih*128+i, p]
  - stationary operand = weight tile [i=128, o=128] (4 per channel)
  - moving operand = xT tile [i=128, p=512]
  - accumulate over ih into a 2-bank PSUM tile [128, 1024] f32, then drain
    to SBUF fused with the bias add (bias varies along PSUM partitions ->
    ACT per-partition activation bias / DVE tensor_scalar). Drains
    alternate ACT/DVE per tile so both engines share every wave.
  - output written transposed; host untransposes.

DMA layout (the kernel is DMA-bound: ~53.5 MB/core over ~716 GB/s per
HBM-stack pair => ~149 us floor):
  - x loads + bulk weight load on the SP HWDGE ring, out stores on the
    ACT HWDGE ring (separate FIFOs -> no head-of-line blocking between
    loads and stores). gpsimd/SWDGE stores crash the device (NRT 101).
  - all 24 channels' weights come in one bulk DMA (split 4/20 so channel
    0's matmuls start ~5 us in, not after the whole 3 MB).

Two precision modes (MODE):
  "fp16": x/w/out cross HBM as fp16 (half traffic); f32 PSUM accumulate,
          bias added in f32. ~3.6e-4 rel err, ~149 us.
  "f32r": all HBM traffic f32; PE runs the full-rate fp32 path (float32r,
          TF32-like). ~1.5e-4 rel err, ~294 us (DMA-bound at f32 bytes).
"""

import numpy as np

MODE = "fp8"  # "fp8" | "fp16" | "f32r"

N_CORES = 8
N_SEL = 192
N_CH = N_SEL // N_CORES  # 24 channels per core
NPT = 2048               # points per channel
CIN = 256
COUT = 256
P = 128                  # SBUF/PSUM partitions
PC = 512                 # moving-operand chunk (one PSUM bank of f32)
W_SPLIT = 4              # channels of weights in the early chunk
X_BUFS = 8
O_BUFS = 8

_CACHE = {}


def _build(mode):
    import concourse.mybir as mybir
    import concourse.tile as tile
    from concourse import bacc

    f32 = mybir.dt.float32
    if mode == "fp8":
        # x crosses HBM as fp8 E3M4 (1 B/elem, 4 mantissa bits -> ~1.3e-2
        # end-to-end rel err on N(0,1) data); w and out stay fp16. The PE
        # runs a mixed-dtype matmul: fp16 stationary x fp8e3 moving.
        x_dt = mybir.dt.float8e3
        w_dt = mybir.dt.float16
        io_dt = mybir.dt.float16   # out dtype
        pe_dt = None
    elif mode == "fp16":
        io_dt = mybir.dt.float16   # dtype of x/w/out in DRAM and SBUF
        pe_dt = mybir.dt.float16   # dtype the PE sees
        x_dt = w_dt = io_dt
    else:
        io_dt = f32
        pe_dt = mybir.dt.float32r
        x_dt = w_dt = io_dt

    def pe_cast(ap):
        return ap.bitcast(pe_dt) if pe_dt is not None and ap.dtype != pe_dt else ap

    nc = bacc.Bacc(None, target_bir_lowering=False)
    # Layouts chosen so every x/out DMA run is >=4 KB contiguous per SBUF
    # partition and the bulk w load ~24 KB: xt/out [n, i, ih, p] and
    # wt [i, n, ih, o] (halves the x/out descriptor count, cuts w's 512 B
    # descriptors 6144 -> 256; ~5 us measured).
    xt_d = nc.dram_tensor("xt", [N_CH, P, 2, NPT], x_dt, kind="ExternalInput")
    wt_d = nc.dram_tensor("wt", [P, N_CH, 2, COUT], w_dt, kind="ExternalInput")
    bt_d = nc.dram_tensor("bt", [2, P, N_CH], f32, kind="ExternalInput")
    out_d = nc.dram_tensor("out", [N_CH, P, 2, NPT], io_dt, kind="ExternalOutput")

    with tile.TileContext(nc) as tc:
        with (
            tc.tile_pool(name="xp", bufs=X_BUFS) as xp,
            tc.tile_pool(name="bp", bufs=1) as bp,
            tc.tile_pool(name="op", bufs=O_BUFS) as op,
            tc.tile_pool(name="ps", bufs=4, space="PSUM") as ps,
        ):
            b_sb = bp.tile([P, 2, N_CH], f32, tag="b")
            w_sb = bp.tile([P, N_CH, 2, COUT], pe_dt or w_dt, tag="w")

            def load_x(n):
                x_sb = xp.tile([P, 2, NPT], pe_dt or x_dt, tag="x")
                nc.sync.dma_start(x_sb[:], pe_cast(xt_d[n]))
                return x_sb

            # Startup: x loads on the SP ring; ALL weight/bias loads on the
            # ACT ring (idle until the first store ~15 us in) so x0 and
            # w[0:4] land in parallel and the PE starts ~1.5 us earlier.
            x_tiles = {0: load_x(0)}
            nc.sync.dma_start(w_sb[:, :W_SPLIT], pe_cast(wt_d[:, :W_SPLIT]))
            nc.sync.dma_start(b_sb[:], bt_d.rearrange("oh o n -> o oh n"))
            x_tiles[1] = load_x(1)
            nc.sync.dma_start(w_sb[:, W_SPLIT:], pe_cast(wt_d[:, W_SPLIT:]))

            for n in range(N_CH):
                x_sb = x_tiles.pop(n) if n in x_tiles else load_x(n)
                o_sb = op.tile([P, 2, NPT], io_dt, tag="o")
                # Tile order (oh, pch): finish all of oh=0 first so its
                # half-store can launch while oh=1 is still computing.
                for tile_idx, (oh, pch) in enumerate(
                    [(0, 0), (0, 1), (1, 0), (1, 1)]
                ):
                    bias_ap = b_sb[:, oh, n : n + 1]
                    acc = ps.tile([P, 2 * PC], f32, tag="acc")
                    for pc2 in range(2):
                        pcg = pch * 2 + pc2
                        for ih in range(2):
                            nc.tensor.matmul(
                                acc[:, pc2 * PC : (pc2 + 1) * PC],
                                w_sb[:, n, ih, oh * P : (oh + 1) * P],
                                x_sb[:, ih, pcg * PC : (pcg + 1) * PC],
                                start=(ih == 0),
                                stop=(ih == 1),
                            )
                    dst = o_sb[:, oh, pch * 2 * PC : (pch + 1) * 2 * PC]
                    if (n * 4 + tile_idx) % 2 == 0:
                        nc.scalar.activation(
                            dst,
                            acc[:],
                            mybir.ActivationFunctionType.Identity,
                            bias=bias_ap,
                        )
                    else:
                        nc.vector.tensor_scalar_add(dst, acc[:], bias_ap)
                    # Store each oh half as soon as its 2 drains land:
                    # halves split between the ACT and SP HWDGE rings
                    # (the only two) so the store stream is smooth and
                    # both rings serve the stores-only tail.
                    if tile_idx == 1:
                        nc.scalar.dma_start(out_d[n, :, 0], o_sb[:, 0])
                    elif tile_idx == 3:
                        nc.sync.dma_start(out_d[n, :, 1], o_sb[:, 1])

    nc.compile()
    return nc


def _get_nc(mode=None):
    mode = mode or MODE
    if mode not in _CACHE:
        _CACHE[mode] = _build(mode)
    return _CACHE[mode]


def _np_dtypes(mode):
    """(x_dtype, w_dtype, out_dtype) on the numpy side."""
    if mode == "fp8":
        import ml_dtypes

        return ml_dtypes.float8_e3m4, np.float16, np.float16
    if mode == "fp16":
        return np.float16, np.float16, np.float16
    return np.float32, np.float32, np.float32


def make_in_maps(x, weight, bias, indices, t, mode=None):
    mode = mode or MODE
    idx = np.asarray(indices).astype(np.int64)
    t = int(np.asarray(t))
    x_np, w_np, _ = _np_dtypes(mode)

    w_g = np.asarray(weight)[idx, t]   # [192, 256, 256] f32
    b_g = np.asarray(bias)[idx, t, 0]  # [192, 256] f32

    x_full = np.asarray(x)
    if mode == "fp8":
        # E3M4 overflows to inf above +-15.5; clip first (x is N(0,1),
        # max |x| ~ 5.1, so this never bites on real data).
        x_full = np.clip(x_full, -15.5, 15.5)

    in_maps = []
    for c in range(N_CORES):
        s = slice(c * N_CH, (c + 1) * N_CH)
        xt_c = np.ascontiguousarray(
            x_full[s]
            .transpose(0, 2, 1)
            .reshape(N_CH, 2, P, NPT)
            .transpose(0, 2, 1, 3),
            dtype=x_np,
        )
        wt_c = np.ascontiguousarray(
            w_g[s].reshape(N_CH, 2, P, COUT).transpose(2, 0, 1, 3), dtype=w_np
        )
        bt_c = np.ascontiguousarray(b_g[s].T, dtype=np.float32).reshape(2, P, N_CH)
        in_maps.append({"xt": xt_c, "wt": wt_c, "bt": bt_c})
    return in_maps


def assemble_out(results):
    out = np.empty((N_SEL, NPT, COUT), dtype=np.float32)
    for c in range(N_CORES):
        s = slice(c * N_CH, (c + 1) * N_CH)
        out_t = (
            results[c]["out"]
            .astype(np.float32)
            .reshape(N_CH, P, 2, NPT)
            .transpose(0, 2, 1, 3)
            .reshape(N_CH, COUT, NPT)
        )
        out[s] = out_t.transpose(0, 2, 1)
    return out


def kernel(x, weight, bias, indices, t):
    from concourse.bass_utils import run_bass_kernel_spmd

    in_maps = make_in_maps(x, weight, bias, indices, t)
    nc = _get_nc()
    res = run_bass_kernel_spmd(nc, in_maps, core_ids=list(range(N_CORES)))
    return assemble_out(res.results)

